# revision 60
# baseline (speedup 1.0000x reference)
"""Trainium2 Bass kernel for nn_Kernel_Conv (conv3x3+GELU -> per-pixel 19x19
conv -> conv3x3+sigmoid), SPMD over 8 NeuronCores.

Sharding: 8 cores = 2 batches x 4 H-slices (32 output rows each). All inputs
are host-preprocessed into per-core slabs (f16) so the device program is
identical on every core.

Structure (v10, 64us):
- ALL bulk DRAM traffic goes through gpsimd software-DGE (descriptors spread
  over all 16 DMA engines at ~400 GB/s; the HW-DGE queues share one engine
  at ~25 GB/s). All 9 band chunks prefetch upfront; a PE p-state warmup
  chain precedes conv1.
- conv1 emitted transposed: per output row y, out[w, c] = sum_k inp6[k, w] *
  w1[k, c] with k = (dy,dx,ci)+bias-row, so GELU lands directly in the
  xT[w, (row,c)] layout the per-pixel stage needs. Both matmuls use K=98
  (dy2 weights zero-padded) so the PE tile config never changes — mixed
  K=98/K=49 cost a ~60ns reconfig bubble per matmul.
- per-pixel conv: w' split into 4 groups of 32, FOUR kh taps stacked on the
  128 PE rows (k = 4x32, q-major). lhsT comes from xTQall, built by
  partition-shift engine copies on vector; rhs is a [128,50] band slice.
  Windows land at 32-col pitch in PSUM where per-element has_written bits
  do the overlap-add for free (a zero-weight "zeroer" matmul opens each
  row's window); eviction is then a single [16,128] copy per row.
- conv2 is column-tiled into PSUM quadrants with k padded to 128 (zero
  weights), interleaved into the pp loop one block behind; sigmoid evicts
  into [16, 2048] quad tiles shipped as one out-DMA per 4 blocks.
"""

import sys
import types

for _p in ("/opt/trn_rl_repo",):
    if _p not in sys.path:
        sys.path.insert(0, _p)

import numpy as np
import ml_dtypes
from contextlib import ExitStack

# Register the NTFF profile hook shim (harmless if tracing is never used)
try:
    import antenv  # noqa: F401
    if "antenv.axon_hooks" not in sys.modules:
        if "/root/.axon_site" not in sys.path:
            sys.path.insert(0, "/root/.axon_site")
        from trn_agent_boot.trn_boot import _ntff_profile_via_ctypes
        _hook = _ntff_profile_via_ctypes("/opt/axon/libaxon_pjrt.so")
        _mod = types.ModuleType("antenv.axon_hooks")
        _mod.get_axon_ntff_profile_hook = lambda: _hook
        sys.modules["antenv.axon_hooks"] = _mod
        antenv.axon_hooks = _mod
except Exception:
    pass

import concourse.bass as bass
import concourse.tile as tile
from concourse import bacc, mybir
from concourse.bass_utils import run_bass_kernel_spmd

BF16 = np.float16

# ---------------- problem constants (hardcoded per the harness contract) ----
B, C, H, W = 2, 16, 128, 128
KK = 19            # per-pixel kernel size
NCORES = 8
HS = 32            # output rows per core
NY = 36            # y rows per core: [h0-2, h0+34)
NX = 56            # x (conv1 out) rows per core: [h0-11, h0+45)
NIN = 58           # input rows per core: [h0-12, h0+46)
GB = 32            # w' group width (4 groups at PE row-tile-legal bases)
GWB = GB + KK - 1  # 50: skewed window width per group
NGB = W // GB      # 4 groups
NM = 5             # kh quads per tap column (ceil(19/4))
NBLK = NY // 4     # 9 per-pixel blocks of 4 y rows
CHUNK_W = 4 * NGB * NM * GWB  # band chunk free els per partition (4000)
NBUF = NBLK        # all 9 band chunks resident (prefetched upfront)
XQW = 52           # xTQ column count (x row slots)
PPW = 146          # merged pp window span per row: out col w maps to 9 + w


def _host_prepare(input, kernel, w1, b1, w2, b2):
    """Build the per-core input maps (all numpy, f16 except b2r)."""
    inp = np.asarray(input, np.float32)
    ker = np.asarray(kernel, np.float32)

    # input, zero-padded: rows [-12, 140), cols [-1, 129)
    inp_pad = np.zeros((B, C, H + 26, W + 2), np.float32)
    inp_pad[:, :, 12:12 + H, 1:1 + W] = inp

    # ker padded along h: rows [-2, 130)
    ker_pad = np.zeros((B, KK * KK, H + 4, W), np.float32)
    ker_pad[:, :, 2:2 + H, :] = ker

    # conv weights as [(dx,c), o] per dy
    def wdy(wmat, dy, order=(0, 1, 2)):
        out = np.zeros((48, 16), np.float32)
        for gi, dx in enumerate(order):
            out[gi * 16:(gi + 1) * 16] = wmat[:, :, dy, dx].T
        return out

    w1f = np.asarray(w1, np.float32)
    # w1s2: col block 0 = mm1 rhs (k=98: dy0 rows 0-47, bias row 48,
    # dy1 rows 49-96, zero row 97); col block 1 rows 0-48 = mm2 rhs (dy2).
    w1s2 = np.zeros((98, 32), np.float32)
    w1s2[0:48, 0:16] = wdy(w1f, 0)
    w1s2[48, 0:16] = np.asarray(b1, np.float32)
    w1s2[49:97, 0:16] = wdy(w1f, 1)
    w1s2[0:48, 16:32] = wdy(w1f, 2)
    w1s2 = np.ascontiguousarray(w1s2).astype(BF16)

    # conv2 weights padded to k=128; dx blocks at 32-aligned partition bases
    # (0: dx=1 direct, 32: dx=0 right-shifted copy, 64: dx=2 left-shifted)
    w2f = np.asarray(w2, np.float32)
    w2s = np.zeros((128, 48), np.float32)
    for dy in range(3):
        blk48 = wdy(w2f, dy, order=(1, 0, 2))
        w2s[0:16, dy * 16:(dy + 1) * 16] = blk48[0:16]
        w2s[32:48, dy * 16:(dy + 1) * 16] = blk48[16:32]
        w2s[64:80, dy * 16:(dy + 1) * 16] = blk48[32:48]
    w2s = np.ascontiguousarray(w2s).astype(BF16)

    # conv2 bias replicated into each PSUM quadrant's partition range
    b2r = np.zeros((128, 1), np.float32)
    for q in range(4):
        b2r[32 * q:32 * q + 16, 0] = np.asarray(b2, np.float32)

    # compact skewed band F[yg, kh, g, v, j]:
    #   yr = h0-2+yg; w' = GB*g+v; w = GB*g + j - 9; kw = 18-(j-v);
    #   xr = yr+kh-9; value = ker[b, kh*19+kw, yr, w] when all indices valid
    yg_i = np.arange(NY)[:, None, None, None, None]
    kh_i = np.arange(KK)[None, :, None, None, None]
    g_i = np.arange(NGB)[None, None, :, None, None]
    v_i = np.arange(GB)[None, None, None, :, None]
    j_i = np.arange(GWB)[None, None, None, None, :]
    kw_i = 18 - (j_i - v_i)
    w_i = GB * g_i + j_i - 9
    valid = (kw_i >= 0) & (kw_i < KK) & (w_i >= 0) & (w_i < W)
    kw_c = np.clip(kw_i, 0, KK - 1)
    w_c = np.clip(w_i, 0, W - 1)

    in_maps = []
    for cid in range(NCORES):
        b = cid // 4
        h0 = 32 * (cid % 4)

        # dx-tripled input slab + ones bias row: [49, NIN, W]; inp6 stacks a
        # row-shifted copy on partitions 49-97 so dy=0,1 fuse into one matmul.
        inp3 = np.zeros((49, NIN, W), np.float32)
        rows = inp_pad[b, :, h0: h0 + NIN, :]  # [C, NIN, W+2]
        for dx in range(3):
            inp3[dx * 16:dx * 16 + 16, :, :] = rows[:, :, dx:dx + W]
        inp3[48] = 1.0
        inp6 = np.zeros((98, NIN, W), np.float32)
        inp6[0:49] = inp3
        inp6[49:98, :NIN - 1] = inp3[:, 1:]
        inp6 = inp6.astype(BF16)

        # band for this core
        yr_i = h0 - 2 + yg_i                 # global y row
        xr_i = yr_i + kh_i - 9               # global x row feeding this tap
        v_ok = valid & (yr_i >= 0) & (yr_i < H) & (xr_i >= 0) & (xr_i < H)
        yr_c = np.clip(yr_i, 0, H - 1)
        p_i = kh_i * KK + kw_c
        F = ker_pad[b, :, 2:2 + H, :][p_i, yr_c, w_c] * v_ok  # [NY,19,4,32,50]
        # stack 4 kh taps on partitions, q-major: band[p=32q+v, (r,g,m,j)] =
        # F[4blk+r, 4m+q, g, v, j]  (q-major so xTQ stacking is a plain
        # 32-partition-block shift, doable as engine copies)
        Fp = np.zeros((NY, 20, NGB, GB, GWB), np.float32)
        Fp[:, :KK] = F
        Q = Fp.reshape(NBLK, 4, NM, 4, NGB, GB, GWB)  # [blk,r,m,q,g,v,j]
        Q = Q.transpose(0, 3, 5, 1, 4, 2, 6)          # [blk,q,v,r,g,m,j]
        bandF = np.ascontiguousarray(Q, dtype=BF16)

        in_maps.append({
            "inp6": np.ascontiguousarray(inp6.reshape(98, NIN * W)),
            "bandF": bandF.reshape(NBLK * 128 * CHUNK_W),
            "w1s": w1s2,
            "w2s": w2s,
            "b2r": b2r,
        })
    return in_maps


def _build_program():
    nc = bacc.Bacc("TRN2", target_bir_lowering=False, debug=False,
                   num_devices=NCORES)
    dt = mybir.dt

    inp6_d = nc.dram_tensor("inp6", [98, NIN * W], dt.float16,
                            kind="ExternalInput").ap()
    bandF_d = nc.dram_tensor("bandF", [NBLK * 128 * CHUNK_W], dt.float16,
                             kind="ExternalInput").ap()
    w1s_d = nc.dram_tensor("w1s", [98, 32], dt.float16,
                           kind="ExternalInput").ap()
    w2s_d = nc.dram_tensor("w2s", [128, 48], dt.float16,
                           kind="ExternalInput").ap()
    b2r_d = nc.dram_tensor("b2r", [128, 1], dt.float32,
                           kind="ExternalInput").ap()
    out_d = nc.dram_tensor("out", [16, HS * W], dt.float32,
                           kind="ExternalOutput").ap()


    with tile.TileContext(nc) as tc:
        with ExitStack() as ctx:
            _body(ctx, tc, inp6_d, bandF_d, w1s_d, w2s_d, b2r_d, out_d)
    nc.compile()
    return nc


def _body(ctx, tc, inp6_d, bandF_d, w1s_d, w2s_d, b2r_d, out_d):
    nc = tc.nc
    dt = mybir.dt
    AFT = mybir.ActivationFunctionType
    ALU = mybir.AluOpType

    consts = ctx.enter_context(tc.tile_pool(name="consts", bufs=1))
    bigs = ctx.enter_context(tc.tile_pool(name="bigs", bufs=1))
    outp = ctx.enter_context(tc.tile_pool(name="outp", bufs=2))
    ps_c1 = ctx.enter_context(tc.tile_pool(name="ps_c1", bufs=2, space="PSUM"))
    ps_pp = ctx.enter_context(tc.tile_pool(name="ps_pp", bufs=4, space="PSUM"))
    ps_c2 = ctx.enter_context(tc.tile_pool(name="ps_c2", bufs=2, space="PSUM"))

    # ---- persistent SBUF tiles -------------------------------------------
    w1s_t = consts.tile([98, 32], dt.float16, tag="w1s")
    w2s_t = consts.tile([128, 48], dt.float16, tag="w2s")
    b2r_t = consts.tile([128, 1], dt.float32, tag="b2r")
    inp6_t = bigs.tile([98, NIN * W], dt.float16, tag="inp6")
    xT_t = bigs.tile([128, NX * 16], dt.float16, tag="xT")
    # all 4 w' groups in one tile, g-major along free dim
    xTQall = bigs.tile([128, NGB * XQW * 16], dt.float16, tag="xTQ")
    y3_t = bigs.tile([128, NY * W], dt.float16, tag="y3")
    zeros_t = bigs.tile([128, 512], dt.float16, tag="zeros")
    band = [bigs.tile([128, CHUNK_W], dt.float16, tag=f"band{i}",
                      name=f"band{i}") for i in range(NBUF)]

    y3_v = y3_t[:].rearrange("p (r w) -> p r w", r=NY)

    # ---- loads + one-time zeroing ----------------------------------------
    # All bulk DRAM traffic goes through gpsimd (software DGE): its
    # descriptors round-robin across all 16 DMA engines, while the HW DGE
    # queues all funnel through a single DMA engine (~25 GB/s total).
    nc.scalar.dma_start(w1s_t[:], w1s_d)
    nc.scalar.dma_start(w2s_t[:], w2s_d)
    nc.scalar.dma_start(b2r_t[:], b2r_d)
    # two halves so conv1 can start on the first rows sooner
    HIN = 30 * W
    nc.gpsimd.dma_start(inp6_t[:, 0:HIN], inp6_d[:, 0:HIN])
    nc.gpsimd.dma_start(inp6_t[:, HIN:], inp6_d[:, HIN:])
    # zero y3 once: edge cols stay zero; partitions 48-127 are the k-pad for
    # conv2 (their weights are zero, but NaN garbage would still poison)
    HY = NY * W // 2
    nc.vector.memset(zeros_t[:], 0.0)
    nc.vector.memset(y3_t[:, 0:HY], 0.0)
    nc.vector.memset(y3_t[:, HY:], 0.0)

    bandF_v = bandF_d.rearrange("(blk p c) -> blk p c", blk=NBLK, p=128,
                                c=CHUNK_W)

    RQ = NGB * NM * GWB  # 1000: chunk els per r-slot
    def band_chunk_dma(blk):
        lo = RQ if blk == 0 else 0          # (0, r=0) = y row 0: never read
        hi = 3 * RQ if blk == NBLK - 1 else CHUNK_W  # (8, r=3) unused too
        nc.gpsimd.dma_start(band[blk][:, lo:hi], bandF_v[blk, :, lo:hi])

    # all band chunks stream back-to-back (xTQ is engine copies, not DMA,
    # so nothing queues behind the band on the software-DGE queue)
    for blk in range(NBLK):
        band_chunk_dma(blk)

    # PE p-state warmup: dummy zero matmuls so conv1 runs at full clock (the
    # PE needs >3us of continuous work to leave the low p-state); sized to
    # end roughly when inp6 lands so the PE never idles before conv1
    warm = ps_c2.tile([128, 512], dt.float32, tag="c2", name="warm")
    for i in range(14):
        nc.tensor.matmul(warm[:, 0:512], zeros_t[:, 0:128],
                         zeros_t[:, 0:512], start=True, stop=True)

    # ---- conv1 + GELU, transposed: xT[w, (row,c)] ------------------------
    # Both matmuls use K=98 (w1s rows 49-97 of the dy2 block are zero) so
    # the PE tile config stays (128, 128) for every instruction — mixed
    # K=98/K=49 tile sizes cost a reconfig bubble per matmul.
    # Flat inp6 slices keep read ranges precise: conv1's first rows only
    # wait on the first inp6 DMA half.
    for tb in range(NX // 8):
        ps = ps_c1.tile([128, 128], dt.float32, tag="c1")
        for rr in range(8):
            i = 8 * tb + rr
            nc.tensor.matmul(ps[:, 16 * rr:16 * rr + 16],
                             inp6_t[:, i * W:i * W + W],
                             w1s_t[:, 0:16],
                             start=True, stop=False)
            nc.tensor.matmul(ps[:, 16 * rr:16 * rr + 16],
                             inp6_t[:, (i + 2) * W:(i + 2) * W + W],
                             w1s_t[:, 16:32],
                             start=False, stop=True)
        nc.scalar.activation(xT_t[:, 128 * tb:128 * (tb + 1)], ps[:],
                             AFT.Gelu)

    # ---- xTQ: partition-stack 4 consecutive x rows per w' group ----------
    # xTQall[32q+v, 832g+e] = xT[32g+v, 16q+e]: a 32-partition-block shift,
    # done as engine copies on the (otherwise idle) scalar+vector engines so
    # nothing queues behind the band chunks on the DMA queue. Column halves
    # let the first-half copies run while conv1's last tiles still compute.
    GW = XQW * 16  # 832
    HA = 416       # first-half columns (jj 0..25)
    # all on vector: it idles during conv1, while scalar runs the GELU
    # evictions (copies there would FIFO-block behind them)
    for e0, e1 in [(0, HA), (HA, GW)]:
        for g in range(NGB):
            for q in range(4):
                dst = xTQall[32 * q:32 * q + 32, GW * g + e0:GW * g + e1]
                src = xT_t[32 * g:32 * g + 32, 16 * q + e0:16 * q + e1]
                nc.vector.tensor_copy(dst, src)

    # ---- conv2 (column-tiled, k padded to 128, interleaved below) --------
    # sigmoid evicts into a [16, 2048] quad tile (partition-shifted down to
    # rows 0-15) so four blocks ship as ONE out DMA
    c2ps = [None]
    othold = [None]

    def emit_conv2(b):
        q = b % 4
        if q == 0:
            c2ps[0] = ps_c2.tile([128, 512], dt.float32, tag="c2",
                                 name=f"c2ps{b}")
            othold[0] = outp.tile([16, 4 * 512], dt.float32, tag="o",
                                  name=f"ot{b // 4}")
        ps = c2ps[0]
        ot = othold[0]
        for dy in range(3):
            nc.tensor.matmul(
                ps[32 * q:32 * q + 16, :],
                w2s_t[:, dy * 16:(dy + 1) * 16],
                y3_v[:, 4 * b + 1 + dy: 4 * b + 5 + dy, :],
                start=(dy == 0), stop=(dy == 2),
                tile_position=(0, 32 * q))
        nc.scalar.activation(ot[0:16, 512 * q:512 * (q + 1)],
                             ps[32 * q:32 * q + 16, :],
                             AFT.Sigmoid, bias=b2r_t[32 * q:32 * q + 16, :])
        if q == 3:
            B = b // 4
            nc.gpsimd.dma_start(
                out_d[:, 2048 * B:2048 * (B + 1)], ot[0:16, :])

    # ---- per-pixel conv: 4 kh taps stacked on k, compact skewed rhs ------
    # Merged overlap-add in PSUM: per row-chain, a zeroer matmul (start=True,
    # zero weights x zero rhs) opens the [0, PPW) window writing 0s and
    # setting every has_written bit; the 20 real matmuls then land at
    # 32-pitch (out col = w + 9), overlaps accumulating per-element.
    def live(blk, r):
        # y rows 0 (blk 0, r 0) and 35 (blk 8, r 3) are never read by conv2
        return 1 <= 4 * blk + r <= 34

    for blk in range(NBLK):
        bt = band[blk % NBUF]
        # full 2 KiB bank per tile: a matmul `start` zeroes has_written for
        # the whole bank, so tiles must never share one
        pp = ps_pp.tile([128, 512], dt.float32, tag="pp")
        for r in range(4):
            if not live(blk, r):
                continue
            nc.tensor.matmul(
                pp[32 * r:32 * r + 16, 0:PPW],
                zeros_t[:, 0:16], zeros_t[:, 0:PPW],
                start=True, stop=False,
                tile_position=(0, 32 * r))
            for g in range(NGB):
                for m in range(NM):
                    jj = 4 * blk + r + 4 * m
                    off = ((r * NGB + g) * NM + m) * GWB
                    nc.tensor.matmul(
                        pp[32 * r:32 * r + 16, 32 * g:32 * g + GWB],
                        xTQall[:, (XQW * g + jj) * 16:(XQW * g + jj) * 16 + 16],
                        bt[:, off:off + GWB],
                        start=False, stop=(g == NGB - 1 and m == NM - 1),
                        tile_position=(0, 32 * r))
        # evict: single copy per row (overlap-add already done in PSUM);
        # split across vector and scalar so neither engine gates the PSUM
        # buffer recycling
        for r in range(4):
            if not live(blk, r):
                continue
            yg = 4 * blk + r
            if r < 2:
                nc.vector.tensor_copy(
                    y3_v[0:16, yg, :], pp[32 * r:32 * r + 16, 9:9 + W])
            else:
                nc.scalar.activation(
                    y3_v[0:16, yg, :], pp[32 * r:32 * r + 16, 9:9 + W],
                    AFT.Copy)

        # dx shifts for conv2 as scalar-engine copies into 32-aligned slots
        nc.scalar.activation(y3_v[32:48, 4 * blk: 4 * blk + 4, 1:W],
                             y3_v[0:16, 4 * blk: 4 * blk + 4, 0:W - 1],
                             AFT.Copy)
        nc.scalar.activation(y3_v[64:80, 4 * blk: 4 * blk + 4, 0:W - 1],
                             y3_v[0:16, 4 * blk: 4 * blk + 4, 1:W],
                             AFT.Copy)
        if blk >= 1:
            emit_conv2(blk - 1)



_NC_CACHE = None
LAST = {}


def _get_nc():
    global _NC_CACHE
    if _NC_CACHE is None:
        _NC_CACHE = _build_program()
    return _NC_CACHE


def kernel(input, kernel, w1, b1, w2, b2, _trace=False, _tmpdir=None):
    in_maps = _host_prepare(input, kernel, w1, b1, w2, b2)
    nc = _get_nc()
    res = run_bass_kernel_spmd(nc, in_maps, core_ids=list(range(NCORES)),
                               trace=_trace, tmpdir=_tmpdir)
    out = np.zeros((B, C, H, W), np.float32)
    for cid in range(NCORES):
        b = cid // 4
        h0 = 32 * (cid % 4)
        out[b, :, h0:h0 + HS, :] = res.results[cid]["out"].reshape(16, HS, W)
    LAST["exec_ns"] = res.exec_time_ns
    LAST["trace"] = res.instructions_and_trace
    return out

